# revision 34
# baseline (speedup 1.0000x reference)
"""MoE routed conv for Trainium2, 8-core SPMD — 1D Winograd F(2,3) variant.

Math: each batch image selects one expert (argmax of scores); output equals
a 3x3 pad-1 conv with the selected expert's filter. We compute only that
expert's conv (5x less work), data-parallel 4 images per core.

Device kernel: Winograd F(2,3) along the width axis cuts PE columns to 2/3
of direct implicit GEMM (12 vs 18 column-units per image):
  For output column pair (2g, 2g+1), with padded input cols d0..d3 =
  xpad[2g..2g+3] and per-kh weight taps w0,w1,w2:
    D0=d0-d2  D1=d1+d2  D2=d2-d1  D3=d1-d3          (input transform, GpSimd)
    Wt0=w0  Wt1=(w0+w1+w2)/2  Wt2=(w0-w1+w2)/2  Wt3=w2   (host)
    M_j[co,h,g] = sum_kh sum_ci Wt_j(kh) D_j(h+kh-1,g)   (PE, PSUM accum)
    out(2g)   = M0+M1+M2                              (2 DVE passes)
    out(2g+1) = M1-M2-M3                              (2 DVE passes)
Loop order j-outer/kh/chunk keeps the stationary weight for 4 consecutive
matmuls and holds only 4-8 PSUM banks live; the transform engines (GpSimd
in, DVE out) run concurrently with the PE.
"""
import numpy as np

B, C, H, W = 32, 128, 56, 56
E, OC = 5, 128
NCORES = 8
IPC = B // NCORES          # images per core
WP = W + 2                 # padded width
G = W // 2                 # 28 column pairs
CH2 = 14                   # output rows per PSUM chunk
NCH = H // CH2             # 4

_program = None


def _build_program():
    import concourse.bacc as bacc
    import concourse.tile as tile
    from concourse.tile import add_dep_helper
    from concourse import mybir

    dt = mybir.dt
    idt = dt.bfloat16
    nc = bacc.Bacc("TRN2", target_bir_lowering=False, debug=False)
    # d: host-side Winograd input transform, [img, ci, j, h, group]
    d_d = nc.dram_tensor("d", [IPC, C, 4, H, G], idt, kind="ExternalInput").ap()
    w_d = nc.dram_tensor("w", [IPC, C, 12, OC], idt, kind="ExternalInput").ap()
    o_d = nc.dram_tensor("o", [IPC, OC, H, W], idt, kind="ExternalOutput").ap()

    NXT = 2

    with tile.TileContext(nc) as tc:
        with (
            tc.tile_pool(name="dp", bufs=1) as dp,
            tc.tile_pool(name="wpool", bufs=1) as wpool,
            tc.tile_pool(name="opool", bufs=1) as opool,
            tc.tile_pool(name="tpool", bufs=16) as tpool,
            tc.tile_pool(name="ps", bufs=8, space="PSUM") as psp,
        ):
            dts = [dp.tile([C, 4, H, G], idt, name=f"dt{i}") for i in range(NXT)]
            wts = [wpool.tile([C, 12, OC], idt, name=f"wt{i}") for i in range(IPC)]
            ots = [opool.tile([OC, H, W], idt, name=f"ot{i}") for i in range(2)]

            anchor = None
            for img in range(IPC):
                dtile = dts[img % NXT]
                wt = wts[img]
                ot = ots[img % 2]
                loads = []
                if img == 0:
                    loads.append(nc.sync.dma_start(out=wt[:, 0:1, :],
                                                   in_=w_d[img, :, 0:1, :]))
                    loads.append(nc.sync.dma_start(out=wt[:, 1:12, :],
                                                   in_=w_d[img, :, 1:12, :]))
                    # j-major stream: j0 first so matmuls start early;
                    # j3 in parallel on the sync queue
                    for j in range(3):
                        loads.append(nc.scalar.dma_start(
                            out=dtile[:, j], in_=d_d[img, :, j]))
                    loads.append(nc.sync.dma_start(
                        out=dtile[:, 3], in_=d_d[img, :, 3]))
                else:
                    loads.append(nc.sync.dma_start(out=wt[:], in_=w_d[img]))
                    loads.append(nc.scalar.dma_start(out=dtile[:], in_=d_d[img]))
                if img >= 1 and anchor is not None:
                    for ld in loads:
                        add_dep_helper(ld.ins, anchor.ins, sync=True,
                                       reason="delay prefetch past head-critical DMAs")

                # matmuls: j-outer (stationary reused over 4 chunks), kh accum.
                # Output transform (TensorTensor may read only ONE psum input):
                #   Act drains a1=M1, a2=M2 to SBUF bf16
                #   DVE:    t01 = M0(psum) + a1        (after j1)
                #   GpSimd: t12 = a1 - a2              (after j2)
                #   DVE/GpSimd: out_even = t01 + a2    (after j2)
                #   DVE:    out_odd  = t12 - M3(psum)  (after j3)
                pss = {}   # (c, j) -> psum tile
                tmp01 = {}
                tmp12 = {}
                am1 = {}
                am2 = {}
                # last image runs in two chunk-halves end-to-end so the
                # first half's drains/passes/stores overlap the second
                # half's matmuls (the transform chain otherwise runs ~5us
                # past the final matmul)
                halves = [(0, 2), (2, 4)] if img == IPC - 1 else [(0, NCH)]
                for (ha, hb) in halves:
                  for j in range(4):
                    for c in range(ha, hb):
                        pss[(c, j)] = psp.tile([OC, CH2, G], dt.float32,
                                               name=f"ps{img}_{c}_{j}", tag="ps")
                    if j == 3 and img == IPC - 1:
                        # final j-group: chunk-outer so each chunk's
                        # accumulation (and its P4 + store) completes early
                        sweep = [(kh, c) for c in range(ha, hb) for kh in range(3)]
                    else:
                        sweep = [(kh, c) for kh in range(3) for c in range(ha, hb)]
                    for (kh, c) in sweep:
                        r0 = c * CH2
                        hs = max(r0, 1 - kh)
                        he = min(r0 + CH2, H + 1 - kh)
                        rhs = dtile[:, j, hs + kh - 1 : he + kh - 1, :]
                        out = pss[(c, j)][:, hs - r0 : he - r0, :]
                        mm = nc.tensor.matmul(out, wt[:, kh * 4 + j, :], rhs,
                                              start=(kh == 0), stop=(kh == 2))
                        if img == 0 and j == 0 and kh == 0 and c == hb - 1:
                            anchor = mm
                    # output-transform passes that become ready after this j
                    if j == 1:
                        for c in range(ha, hb):
                            a = tpool.tile([OC, CH2, G], idt,
                                           name=f"a1_{img}_{c}", tag="tm")
                            am1[c] = a
                            nc.scalar.activation(a[:], pss[(c, 1)][:],
                                                 mybir.ActivationFunctionType.Copy)
                            t = tpool.tile([OC, CH2, G], idt,
                                           name=f"t01_{img}_{c}", tag="tm")
                            tmp01[c] = t
                            nc.vector.tensor_add(t[:], pss[(c, 0)][:], a[:])
                    elif j == 2:
                        last_half = img == IPC - 1 and ha == 2
                        # last half: start chunk 3's (longest) chain first,
                        # and run its t12 on the faster DVE instead of GpSimd
                        corder = [3, 2] if last_half else range(ha, hb)
                        for c in corder:
                            r0 = c * CH2
                            a = tpool.tile([OC, CH2, G], idt,
                                           name=f"a2_{img}_{c}", tag="tm")
                            am2[c] = a
                            nc.scalar.activation(a[:], pss[(c, 2)][:],
                                                 mybir.ActivationFunctionType.Copy)
                            t = tpool.tile([OC, CH2, G], idt,
                                           name=f"t12_{img}_{c}", tag="tm")
                            tmp12[c] = t
                            teng = nc.vector if (last_half and c == 3) else nc.gpsimd
                            teng.tensor_sub(t[:], am1[c][:], a[:])
                            eng = nc.gpsimd if c < 2 else nc.vector
                            eng.tensor_add(ot[:, r0 : r0 + CH2, 0:56:2],
                                           tmp01[c][:], a[:])
                    elif j == 3:
                        for c in range(ha, hb):
                            r0 = c * CH2
                            nc.vector.tensor_sub(
                                ot[:, r0 : r0 + CH2, 1:56:2],
                                tmp12[c][:], pss[(c, 3)][:])
                            if img == IPC - 1:
                                # merged first store, final store on the idle
                                # scalar queue (parallel descriptor path)
                                if c == 1:
                                    nc.sync.dma_start(out=o_d[img, :, 0:28, :],
                                                      in_=ot[:, 0:28, :])
                                elif c == 2:
                                    nc.sync.dma_start(out=o_d[img, :, 28:42, :],
                                                      in_=ot[:, 28:42, :])
                                elif c == 3:
                                    nc.scalar.dma_start(out=o_d[img, :, 42:56, :],
                                                        in_=ot[:, 42:56, :])
                        if img < IPC - 1:
                            nc.sync.dma_start(out=o_d[img, :, 0:28, :],
                                              in_=ot[:, 0:28, :])
                            nc.sync.dma_start(out=o_d[img, :, 28:56, :],
                                              in_=ot[:, 28:56, :])

    nc.compile()
    return nc


def _get_program():
    global _program
    if _program is None:
        _program = _build_program()
    return _program


def kernel(x: np.ndarray, scores: np.ndarray, weight: np.ndarray,
           **run_kwargs) -> np.ndarray:
    import ml_dtypes
    from concourse.bass_utils import run_bass_kernel_spmd

    x = np.asarray(x, dtype=np.float32)
    scores = np.asarray(scores, dtype=np.float32)
    weight = np.asarray(weight, dtype=np.float32)

    expert = np.argmax(scores, axis=1)                       # [B]
    w_sel = weight.reshape(E, OC, C, 3, 3)[expert]           # [B, co, ci, kh, kw]
    # Winograd weight transform G.w per kh: [B, co, ci, kh, j]
    w0, w1, w2 = w_sel[..., 0], w_sel[..., 1], w_sel[..., 2]
    wt = np.stack([w0, (w0 + w1 + w2) * 0.5, (w0 - w1 + w2) * 0.5, w2], axis=-1)
    # lhsT layout: [ci, kh*4+j, co]
    w_lhsT = np.ascontiguousarray(
        wt.transpose(0, 2, 3, 4, 1).reshape(B, C, 12, OC)).astype(ml_dtypes.bfloat16)
    # host-side Winograd input transform: D[j][b,ci,h,g]
    xpad = np.zeros((B, C, H, WP), np.float32)
    xpad[:, :, :, 1 : W + 1] = x
    d0 = xpad[:, :, :, 0:56:2]
    d1 = xpad[:, :, :, 1:57:2]
    d2 = xpad[:, :, :, 2:58:2]
    d3 = xpad[:, :, :, 3:58:2]
    D = np.stack([d0 - d2, d1 + d2, d2 - d1, d1 - d3],
                 axis=2).astype(ml_dtypes.bfloat16)      # [B, C, 4, H, G]

    nc = _get_program()
    in_maps = [
        {"d": D[k * IPC : (k + 1) * IPC], "w": w_lhsT[k * IPC : (k + 1) * IPC]}
        for k in range(NCORES)
    ]
    res = run_bass_kernel_spmd(nc, in_maps, list(range(NCORES)), **run_kwargs)
    out = np.concatenate([res.results[k]["o"] for k in range(NCORES)], axis=0)
    if run_kwargs:
        kernel.last_results = res
    return out.astype(np.float32)
